# revision 1
# baseline (speedup 1.0000x reference)
"""Trainium2 Bass kernel for a dense transformer block (pre-LN, 6-head causal
attention, 4x FFN) over x:(128,256,384) f32.

Strategy: pure data-parallel over batch across 8 NeuronCores (16 sequences per
core). Per-core Tile kernel computes the whole block per sequence:
  LN1 -> QKV (bf16 matmuls, LN scale/shift folded into weights on host)
  -> causal softmax (no max-subtract; scores are tiny by construction)
  -> P@V via PE transposes -> proj + residual -> LN2 -> FFN (relu) + residual.
All matmul operands are bf16 (fp32 PSUM accumulation); the residual stream
stays fp32 end-to-end. ACT stays on one function table (ln/exp/copy/relu);
rsqrt is computed as exp(-0.5*ln(var+eps)).
"""

import os
import sys

import numpy as np

try:
    import concourse.bass as bass  # noqa: F401
except ImportError:
    sys.path.insert(0, "/opt/trn_rl_repo")

import ml_dtypes
from contextlib import ExitStack

import concourse.bass as bass
import concourse.tile as tile
from concourse import bacc, mybir
from concourse.bass_utils import run_bass_kernel_spmd

BF16 = ml_dtypes.bfloat16

N_CORES = 8
B, T, C = 128, 256, 384
H, DH = 6, 64
F = 4 * C  # 1536
BPC = B // N_CORES  # sequences per core
NT = T // 128  # 2 t-tiles
NC_ = C // 128  # 3 c-chunks
NF = F // 128  # 12 f-chunks
LN_EPS = 1e-5

AF = mybir.ActivationFunctionType
ALU = mybir.AluOpType
F32 = mybir.dt.float32
BF = mybir.dt.bfloat16

_PROGRAM_CACHE = {}
LAST_EXEC_NS = None
LAST_RESULTS = None
PROFILE = bool(int(os.environ.get("KERNEL_PROFILE", "0")))
DMA_T_XN = bool(int(os.environ.get("KERNEL_DMA_T_XN", "0")))
ATPACK = bool(int(os.environ.get("KERNEL_ATPACK", "1")))
DMA_T_P = bool(int(os.environ.get("KERNEL_DMA_T_P", "0")))
TRACE_DIR = os.environ.get("KERNEL_TRACE_DIR") or None


def _bcast_h(ap, n):
    """Insert a stride-0 dim of size n after the partition dim of a 2D AP."""
    return bass.AP(
        tensor=ap.tensor, offset=ap.offset, ap=[ap.ap[0], [0, n], ap.ap[1]]
    )


def _emit_seq(nc, tc, pools, cst, b, x_d, out_d, flags):
    """Emit IR for one sequence b."""
    (wpool, xpool, apool, spool, psA, psB, psS, psT) = pools

    # ---- load x (T-major: partition = t%128) ----
    x_sb = xpool.tile([128, NT, C], F32, tag="x", name="x_sb")
    nc.sync.dma_start(x_sb[:], x_d[b].rearrange("(tt p) c -> p tt c", p=128))

    def layer_norm_to_bf16(src_sb, tag):
        """bn_stats/aggr per t-tile; returns (x - mu) * rstd as bf16.
        rstd = exp(-0.5 * ln(var + eps)) keeps ACT on the ln/exp table."""
        st = spool.tile([128, NT, 6], F32, tag=f"st{tag}", name="st")
        mv = spool.tile([128, NT, 2], F32, tag=f"mv{tag}", name="mv")
        for tt in range(NT):
            nc.vector.bn_stats(st[:, tt], src_sb[:, tt])
            nc.vector.bn_aggr(mv[:, tt], st[:, tt])
        lnv = spool.tile([128, NT], F32, tag=f"lnv{tag}", name="lnv")
        rstd = spool.tile([128, NT], F32, tag=f"rstd{tag}", name="rstd")
        nc.scalar.activation(lnv[:], mv[:, :, 1], AF.Ln, bias=cst["eps"][:, 0:1])
        nc.scalar.activation(rstd[:], lnv[:], AF.Exp, scale=-0.5)
        xn = xpool.tile([128, NT, C], BF, tag=f"xn{tag}", name="xn")
        for tt in range(NT):
            nc.gpsimd.tensor_scalar(
                out=xn[:, tt],
                in0=src_sb[:, tt],
                scalar1=mv[:, tt, 0:1],
                scalar2=rstd[:, tt : tt + 1],
                op0=ALU.subtract,
                op1=ALU.mult,
            )
        return xn

    def transpose_to_cmajor(xn, tag):
        """[128, NT, C] bf16 -> [128, NC_, T] bf16 via PE transposes."""
        xnT = xpool.tile([128, NC_, T], BF, tag=f"xnT{tag}", name="xnT")
        if DMA_T_XN:
            for cc in range(NC_):
                for tt in range(NT):
                    nc.sync.dma_start_transpose(
                        xnT[:, cc, tt * 128 : (tt + 1) * 128],
                        xn[:, tt, cc * 128 : (cc + 1) * 128],
                    )
            return xnT
        tp = psT.tile([128, NC_, T], BF, tag="tps", name="tp")
        for cc in range(NC_):
            for tt in range(NT):
                nc.tensor.transpose(
                    tp[:, cc, tt * 128 : (tt + 1) * 128],
                    xn[:, tt, cc * 128 : (cc + 1) * 128],
                    cst["ident"][:],
                )
        nc.vector.tensor_copy(xnT[:], tp[:])
        return xnT

    xn1 = layer_norm_to_bf16(x_sb, "1")
    xnT = transpose_to_cmajor(xn1, "1")
    yield

    # ---- QKV projections (C-major Q/K, T-major V) ----
    QT = apool.tile([128, NC_, T], BF, tag="QT", name="QT")
    KT = apool.tile([128, NC_, T], BF, tag="KT", name="KT")
    for dst, w_sb, b_sb, b_nz in (
        (QT, cst["wq"], cst["bq"], flags["bq_nz"]),
        (KT, cst["wk"], cst["bk"], flags["bk_nz"]),
    ):
        for dt in range(NC_):
            ps = psA.tile([128, T], F32, tag="psA", name="ps_qk")
            for cc in range(NC_):
                nc.tensor.matmul(
                    ps[:],
                    w_sb[:, cc, dt * 128 : (dt + 1) * 128],
                    xnT[:, cc],
                    start=(cc == 0),
                    stop=(cc == NC_ - 1),
                )
            if b_nz:
                nc.scalar.activation(
                    dst[:, dt], ps[:], AF.Identity, bias=b_sb[:, dt : dt + 1]
                )
            else:
                nc.scalar.copy(dst[:, dt], ps[:])
    V = apool.tile([128, NT, C], BF, tag="V", name="V")
    for tt in range(NT):
        ps = psB.tile([128, C], F32, tag="psB", name="ps_v")
        for cc in range(NC_):
            nc.tensor.matmul(
                ps[:],
                xnT[:, cc, tt * 128 : (tt + 1) * 128],
                cst["wv"][:, cc],
                start=(cc == 0),
                stop=(cc == NC_ - 1),
            )
        nc.scalar.copy(V[:, tt], ps[:])
        if flags["bv_nz"]:
            nc.vector.tensor_add(V[:, tt], V[:, tt], cst["bv_bc"][:])
    yield

    # ---- attention ----
    # E/P layout: [128, H, 384] where cols 0:128 = (t0 x s0), 128:384 = (t1 x s)
    E = apool.tile([128, H, 384], BF, tag="E", name="E")
    P = E  # mask applied in place
    sums = spool.tile([128, 2, H], BF, tag="sums", name="sums")
    rec = spool.tile([128, 2, H], F32, tag="rec", name="rec")
    attnT = apool.tile([128, NC_, T], BF, tag="attnT", name="attnT")

    # scores + exp per head (adjacent emission so K=64 row groups 0/1 pipeline)
    for h in range(H):
        dt, off = h // 2, (h % 2) * 64
        sc = psS.tile([128, 384], F32, tag="sc", name="sc")
        nc.tensor.matmul(
            sc[:, 0:128],
            QT[off : off + 64, dt, 0:128],
            KT[off : off + 64, dt, 0:128],
            start=True,
            stop=True,
        )
        nc.tensor.matmul(
            sc[:, 128:384],
            QT[off : off + 64, dt, 128:256],
            KT[off : off + 64, dt, :],
            start=True,
            stop=True,
        )
        nc.scalar.activation(E[:, h], sc[:], AF.Exp)
        if int(os.environ.get("KERNEL_PAIRSM", "0")) and h % 2 == 1:
            hs = slice(h - 1, h + 1)
            nc.vector.tensor_mul(E[:, hs], E[:, hs], _bcast_h(cst["cmask"], 2))
            with nc.allow_low_precision(reason="softmax row-sums tolerate bf16"):
                nc.vector.reduce_sum(
                    sums[:, 0, hs], P[:, hs, 0:128], axis=mybir.AxisListType.X
                )
                nc.vector.reduce_sum(
                    sums[:, 1, hs], P[:, hs, 128:384], axis=mybir.AxisListType.X
                )
            nc.vector.reciprocal(rec[:, :, hs], sums[:, :, hs])
    yield

    if not int(os.environ.get("KERNEL_PAIRSM", "0")):
        nc.vector.tensor_mul(E[:], E[:], _bcast_h(cst["cmask"], H))
        with nc.allow_low_precision(reason="softmax row-sums tolerate bf16"):
            nc.vector.reduce_sum(sums[:, 0], P[:, :, 0:128], axis=mybir.AxisListType.X)
            nc.vector.reduce_sum(sums[:, 1], P[:, :, 128:384], axis=mybir.AxisListType.X)
        nc.vector.reciprocal(
            rec.rearrange("p a b -> p (a b)"), sums.rearrange("p a b -> p (a b)")
        )

    for pair in range(H // 2):
        ptp = None if DMA_T_P else psT.tile([128, 2, 384], BF, tag="tps", name="ptp")
        pts = apool.tile([128, 2, 384], BF, tag="pts", name="pts")
        for s in range(2):
            h = pair * 2 + s
            # normalize P rows (per-head, per-t-tile scalars; DVE 4x on bf16)
            nc.vector.tensor_scalar(
                out=P[:, h, 0:128], in0=P[:, h, 0:128],
                scalar1=rec[:, 0, h : h + 1], scalar2=None, op0=ALU.mult,
            )
            nc.vector.tensor_scalar(
                out=P[:, h, 128:384], in0=P[:, h, 128:384],
                scalar1=rec[:, 1, h : h + 1], scalar2=None, op0=ALU.mult,
            )
            # transpose P blocks: (t0,s0)->[s0,t0], (t1,s0)->[s0,t1], (t1,s1)->[s1,t1]
            for blk in range(3):
                if DMA_T_P:
                    nc.sync.dma_start_transpose(
                        pts[:, s, blk * 128 : (blk + 1) * 128],
                        P[:, h, blk * 128 : (blk + 1) * 128],
                    )
                else:
                    nc.tensor.transpose(
                        ptp[:, s, blk * 128 : (blk + 1) * 128],
                        P[:, h, blk * 128 : (blk + 1) * 128],
                        cst["ident"][:],
                    )
        if not DMA_T_P:
            nc.vector.tensor_copy(pts[:], ptp[:])
        if ATPACK:
            at = psT.tile([128, 256], F32, tag="tps", name="atp")
            for s in range(2):
                h = pair * 2 + s
                dh = h * DH
                tp_kw = {} if s == 0 else {"tile_position": (0, 64)}
                sl = slice(s * 64, (s + 1) * 64)
                nc.tensor.matmul(
                    at[sl, 0:256], V[:, 0, dh : dh + 64], pts[:, s, 0:256],
                    start=True, stop=False, skip_group_check=True, **tp_kw,
                )
                nc.tensor.matmul(
                    at[sl, 128:256], V[:, 1, dh : dh + 64], pts[:, s, 256:384],
                    start=False, stop=True, skip_group_check=True, **tp_kw,
                )
            if int(os.environ.get("KERNEL_ATCOPY_ACT", "1")):
                nc.scalar.copy(attnT[:, pair], at[:])
            else:
                nc.vector.tensor_copy(attnT[:, pair], at[:])
        else:
            for s in range(2):
                h = pair * 2 + s
                dh = h * DH
                at = psT.tile([64, 256], F32, tag="tps", name="at")
                nc.tensor.matmul(
                    at[:], V[:, 0, dh : dh + 64], pts[:, s, 0:256], start=True, stop=False
                )
                nc.tensor.matmul(
                    at[:, 128:256], V[:, 1, dh : dh + 64], pts[:, s, 256:384],
                    start=False, stop=True,
                )
                nc.vector.tensor_copy(attnT[s * 64 : (s + 1) * 64, pair], at[:])
    yield

    # ---- projection + residual ----
    x2 = xpool.tile([128, NT, C], F32, tag="x2", name="x2")
    for tt in range(NT):
        ps = psB.tile([128, C], F32, tag="psB", name="ps_proj")
        for cc in range(NC_):
            nc.tensor.matmul(
                ps[:],
                attnT[:, cc, tt * 128 : (tt + 1) * 128],
                cst["wp"][:, cc],
                start=(cc == 0),
                stop=(cc == NC_ - 1),
            )
        if int(os.environ.get("KERNEL_RESID_POOL", "0")):
            ycp = spool.tile([128, NT, C], F32, tag="ycp", name="ycp")
            nc.scalar.copy(ycp[:, tt], ps[:])
            nc.gpsimd.tensor_add(x2[:, tt], x_sb[:, tt], ycp[:, tt])
        else:
            nc.vector.tensor_add(x2[:, tt], x_sb[:, tt], ps[:])
        if flags["bp_nz"]:
            nc.vector.tensor_add(x2[:, tt], x2[:, tt], cst["bp_bc"][:])

    # ---- LN2 + FFN ----
    xn2 = layer_norm_to_bf16(x2, "2")
    xn2T = transpose_to_cmajor(xn2, "2")
    yield

    zT = apool.tile([128, NF, T], BF, tag="zT", name="zT")
    if not flags["b1_nz"]:
        # paired f-tiles: one [128,512] psum bank, one relu per pair
        for fp in range(NF // 2):
            ps = psA.tile([128, 2, T], F32, tag="psA", name="ps_z")
            for k in range(2):
                ft = fp * 2 + k
                for cc in range(NC_):
                    nc.tensor.matmul(
                        ps[:, k],
                        cst["w1"][:, cc, ft * 128 : (ft + 1) * 128],
                        xn2T[:, cc],
                        start=(cc == 0),
                        stop=(cc == NC_ - 1),
                    )
            nc.scalar.activation(
                zT[:, fp * 2 : fp * 2 + 2].rearrange("p a b -> p (a b)"),
                ps.rearrange("p a b -> p (a b)"),
                AF.Relu,
            )
    else:
        for ft in range(NF):
            ps = psA.tile([128, T], F32, tag="psA", name="ps_z1")
            for cc in range(NC_):
                nc.tensor.matmul(
                    ps[:],
                    cst["w1"][:, cc, ft * 128 : (ft + 1) * 128],
                    xn2T[:, cc],
                    start=(cc == 0),
                    stop=(cc == NC_ - 1),
                )
            nc.scalar.activation(
                zT[:, ft], ps[:], AF.Relu, bias=cst["b1e"][:, ft : ft + 1]
            )

    yield
    out_sb = xpool.tile([128, NT, C], F32, tag="out", name="out_sb")
    for tt in range(NT):
        ps = psB.tile([128, C], F32, tag="psB", name="ps_o")
        for fc in range(NF):
            nc.tensor.matmul(
                ps[:],
                zT[:, fc, tt * 128 : (tt + 1) * 128],
                cst["w2"][:, fc],
                start=(fc == 0),
                stop=(fc == NF - 1),
            )
        if int(os.environ.get("KERNEL_RESID_POOL", "0")):
            ocp = spool.tile([128, NT, C], F32, tag="ocp", name="ocp")
            nc.scalar.copy(ocp[:, tt], ps[:])
            nc.gpsimd.tensor_add(out_sb[:, tt], x2[:, tt], ocp[:, tt])
        else:
            nc.vector.tensor_add(out_sb[:, tt], x2[:, tt], ps[:])
        if flags["b2_nz"]:
            nc.vector.tensor_add(out_sb[:, tt], out_sb[:, tt], cst["b2_bc"][:])
    nc.sync.dma_start(out_d[b].rearrange("(tt p) c -> p tt c", p=128), out_sb[:])
    yield


def _build_program(flags):
    nc = bacc.Bacc("TRN2", target_bir_lowering=False, debug=False)

    x_d = nc.dram_tensor("x_shard", (BPC, T, C), F32, kind="ExternalInput")
    out_d = nc.dram_tensor("out", (BPC, T, C), F32, kind="ExternalOutput")
    w_specs = {
        "wq": (128, NC_, C), "wk": (128, NC_, C), "wv": (128, NC_, C),
        "wp": (128, NC_, C), "w1": (128, NC_, F), "w2": (128, NF, C),
        "ident": (128, 128), "cmask": (128, 384),
    }
    b_specs = {"bq": (128, NC_), "bk": (128, NC_), "b1e": (128, NF)}
    if flags["bv_nz"]:
        b_specs["bv_bc"] = (128, C)
    if flags["bp_nz"]:
        b_specs["bp_bc"] = (128, C)
    if flags["b2_nz"]:
        b_specs["b2_bc"] = (128, C)
    dram = {}
    for name, shape in w_specs.items():
        dram[name] = nc.dram_tensor(name, shape, BF, kind="ExternalInput")
    for name, shape in b_specs.items():
        dram[name] = nc.dram_tensor(name, shape, F32, kind="ExternalInput")

    with tile.TileContext(nc) as tc, ExitStack() as ctx:
        wpool = ctx.enter_context(tc.tile_pool(name="weights", bufs=1))
        xpool = ctx.enter_context(tc.tile_pool(name="xpool", bufs=int(os.environ.get("KERNEL_SBUFS", "4"))))
        apool = ctx.enter_context(tc.tile_pool(name="apool", bufs=int(os.environ.get("KERNEL_SBUFS", "4"))))
        spool = ctx.enter_context(tc.tile_pool(name="spool", bufs=int(os.environ.get("KERNEL_SBUFS", "4"))))
        pb = [int(v) for v in os.environ.get("KERNEL_PSUM", "2,2,2,2").split(",")]
        psA = ctx.enter_context(tc.tile_pool(name="psA", bufs=pb[0], space="PSUM"))
        psB = ctx.enter_context(tc.tile_pool(name="psB", bufs=pb[1], space="PSUM"))
        psS = ctx.enter_context(tc.tile_pool(name="psS", bufs=pb[2], space="PSUM"))
        psT = ctx.enter_context(tc.tile_pool(name="psT", bufs=pb[3], space="PSUM"))

        cst = {}
        heavy = ("wp", "w1", "w2") if int(os.environ.get("KERNEL_DEFER_W", "1")) else ()
        mid = ("wq", "wk", "wv") if int(os.environ.get("KERNEL_MID_QKV", "1")) else ()
        heavy = heavy + tuple(n for n in mid if n not in heavy)
        for name in dram:
            t = wpool.tile(list(dram[name].shape), dram[name].dtype, tag=name, name=name + "_sb")
            if name not in heavy:
                nc.sync.dma_start(t[:], dram[name][:])
            cst[name] = t
        def _load_heavy():
            for name in heavy:
                if name in dram and name not in mid:
                    nc.sync.dma_start(cst[name][:], dram[name][:])

        def _load_mid():
            for name in mid:
                nc.sync.dma_start(cst[name][:], dram[name][:])
        eps = wpool.tile([128, 1], F32, tag="eps", name="eps_sb")
        nc.vector.memset(eps[:], LN_EPS)
        cst["eps"] = eps

        pools = (wpool, xpool, apool, spool, psA, psB, psS, psT)
        depth = int(os.environ.get("KERNEL_ILV", "6"))
        stagger = int(os.environ.get("KERNEL_STAGGER", "1"))
        gens = [
            _emit_seq(nc, tc, pools, cst, b, x_d, out_d, flags)
            for b in range(BPC)
        ]
        live = []
        nxt = 0
        rnd = 0
        heavy_loaded = False
        mid_loaded = False
        while live or nxt < BPC:
            if rnd == 1 and not mid_loaded:
                _load_mid()
                mid_loaded = True
            if rnd == 2 and not heavy_loaded:
                _load_heavy()
                heavy_loaded = True
            if nxt < BPC and len(live) < depth and rnd % max(stagger, 1) == 0:
                live.append(nxt)
                nxt += 1
            for b in list(live):
                if next(gens[b], "end") == "end":
                    live.remove(b)
            rnd += 1

    nc.compile()
    _dedupe_act_table_loads(nc)
    return nc


def _dedupe_act_table_loads(nc):
    """All ACT funcs in this kernel live in one act table; rewrite the
    auto-inserted per-function table loads to a single load of that table."""
    if not int(os.environ.get("KERNEL_ACT_DEDUP", "1")):
        return
    try:
        from concourse.hw_specs import get_activation_tables

        tabs = get_activation_tables(nc.m.arch)
        need = {AF.Exp, AF.Ln, AF.Relu, AF.Copy, AF.Identity}
        combined = next(
            i for i, fs in enumerate(tabs.values()) if need <= fs
        )
    except Exception:
        return
    for blk in nc.m.functions[0].blocks:
        first = True
        keep = []
        for inst in blk.instructions:
            if isinstance(inst, mybir.InstLoadActFuncSet) and inst.sync_info is None:
                if first:
                    inst.act_func_set_id = combined
                    first = False
                    keep.append(inst)
                continue
            keep.append(inst)
        if len(keep) != len(blk.instructions):
            blk.instructions[:] = keep


def _prepare_host_inputs(x, Wq, Wk, Wv, Wp, bp, W1, b1, W2, b2, g1, be1, g2, be2):
    f = np.float32
    x = np.asarray(x, f)
    Wq = np.asarray(Wq, f)
    Wk = np.asarray(Wk, f)
    Wv = np.asarray(Wv, f)
    Wp = np.asarray(Wp, f)
    W1 = np.asarray(W1, f)
    W2 = np.asarray(W2, f)
    bp = np.asarray(bp, f)
    b1 = np.asarray(b1, f)
    b2 = np.asarray(b2, f)
    g1 = np.asarray(g1, f)
    be1 = np.asarray(be1, f)
    g2 = np.asarray(g2, f)
    be2 = np.asarray(be2, f)

    # stack per-head QKV weights: (H, C, DH) -> (C, C) with d = h*DH + dh
    Wq_all = np.transpose(Wq, (1, 0, 2)).reshape(C, C)
    Wk_all = np.transpose(Wk, (1, 0, 2)).reshape(C, C)
    Wv_all = np.transpose(Wv, (1, 0, 2)).reshape(C, C)

    scale = 1.0 / np.sqrt(np.float32(C))
    Wq_eff = (g1[:, None] * Wq_all) * scale
    bq = (be1 @ Wq_all) * scale
    Wk_eff = g1[:, None] * Wk_all
    bk = be1 @ Wk_all
    Wv_eff = g1[:, None] * Wv_all
    bv = be1 @ Wv_all
    W1_eff = g2[:, None] * W1
    b1e = b1 + be2 @ W1

    def chunk_k(w, nk):  # (K, N) -> (128, nk, N)
        K, N = w.shape
        return np.ascontiguousarray(
            w.reshape(nk, 128, N).transpose(1, 0, 2).astype(BF16)
        )

    flags = {
        "bq_nz": bool(np.any(bq != 0)),
        "bk_nz": bool(np.any(bk != 0)),
        "bv_nz": bool(np.any(bv != 0)),
        "bp_nz": bool(np.any(bp != 0)),
        "b1_nz": bool(np.any(b1e != 0)),
        "b2_nz": bool(np.any(b2 != 0)),
    }
    tri = np.tril(np.ones((128, 128), np.float32))
    cmask = np.concatenate([tri, np.ones((128, 128), np.float32), tri], axis=1)

    common = {
        "wq": chunk_k(Wq_eff, NC_),
        "wk": chunk_k(Wk_eff, NC_),
        "wv": chunk_k(Wv_eff, NC_),
        "wp": chunk_k(Wp, NC_),
        "w1": chunk_k(W1_eff, NC_),
        "w2": chunk_k(W2, NF),
        "ident": np.eye(128, dtype=np.float32).astype(BF16),
        "cmask": np.ascontiguousarray(cmask.astype(BF16)),
        "bq": np.ascontiguousarray(bq.reshape(NC_, 128).T.astype(f)),
        "bk": np.ascontiguousarray(bk.reshape(NC_, 128).T.astype(f)),
        "b1e": np.ascontiguousarray(b1e.reshape(NF, 128).T.astype(f)),
    }
    if flags["bv_nz"]:
        common["bv_bc"] = np.ascontiguousarray(np.broadcast_to(bv, (128, C)).astype(f))
    if flags["bp_nz"]:
        common["bp_bc"] = np.ascontiguousarray(np.broadcast_to(bp, (128, C)).astype(f))
    if flags["b2_nz"]:
        common["b2_bc"] = np.ascontiguousarray(np.broadcast_to(b2, (128, C)).astype(f))
    return x, common, flags


def kernel(x, Wq, Wk, Wv, Wp, bp, W1, b1, W2, b2, g1, be1, g2, be2):
    global LAST_EXEC_NS, LAST_RESULTS
    x, common, flags = _prepare_host_inputs(
        x, Wq, Wk, Wv, Wp, bp, W1, b1, W2, b2, g1, be1, g2, be2
    )
    key = tuple(sorted(flags.items()))
    if key not in _PROGRAM_CACHE:
        _PROGRAM_CACHE[key] = _build_program(flags)
    nc = _PROGRAM_CACHE[key]

    in_maps = []
    for c in range(N_CORES):
        m = dict(common)
        m["x_shard"] = np.ascontiguousarray(x[c * BPC : (c + 1) * BPC])
        in_maps.append(m)

    kwargs = {}
    if PROFILE:
        kwargs["trace"] = True
        if TRACE_DIR:
            kwargs["tmpdir"] = TRACE_DIR
    res = run_bass_kernel_spmd(nc, in_maps, core_ids=list(range(N_CORES)), **kwargs)
    LAST_EXEC_NS = res.exec_time_ns
    LAST_RESULTS = res
    out = np.concatenate([np.asarray(r["out"]) for r in res.results], axis=0)
    return out.astype(np.float32)

